# revision 9
# baseline (speedup 1.0000x reference)
"""PhiQ kernel for Trainium2: batched entanglement entropy via
PE Gram matmuls + one-sided Jacobi SVD on the vector engine.

Self-contained: kernel(sites) -> (8192,) float32.
Data-parallel over 8 cores (1024 batch each).
"""
import numpy as np
import concourse.bass as bass
import concourse.bacc as bacc
import concourse.mybir as mybir
import concourse.tile as tile
from concourse import masks
from concourse.ap import AP
from concourse.bass_utils import run_bass_kernel_spmd

F32 = mybir.dt.float32
ALU = mybir.AluOpType
ACT = mybir.ActivationFunctionType

NCORES = 8
B = 1024            # batch per core
NG = 128            # gram groups of 8 batches
NS, D = 16, 256
P = 128

N_SWEEP_W = 4       # 8x8 whole problem
N_SWEEP_LR = 3      # 4x4 left/right problems


def _hap(t_ap, extra_off, dims):
    """hand-built AP on the same backing tensor as t_ap"""
    return AP(t_ap.tensor, t_ap.offset + extra_off, dims)


def _angle_ops(nc, wp, m, GT, NT, a_off, a_dims, b_off, b_dims, tag):
    """Compute rotation params from gamma tile GT (128,m) and norms tile NT
    slices (alpha at a_off/a_dims, beta at b_off/b_dims).
    Returns (T, C) tiles (128, m): tangent and cosine. Also updates norms:
    alpha -= t*gamma, beta += t*gamma."""
    w = lambda tg: wp.tile([P, m], F32, tag=f"{tag}{tg}", name=f"{tag}{tg}")
    nta = NT[:]
    alpha = _hap(nta, a_off, a_dims)
    beta = _hap(nta, b_off, b_dims)
    DL = w("dl"); G2 = w("g2"); R2 = w("r2"); Z = w("z"); NZ = w("nz")
    AZ = w("az"); SQ = w("sq"); RHO = w("rho"); DEN = w("den"); RD = w("rd")
    SGN = w("sgn"); T = w("t"); T2 = w("t2"); A1 = w("a1"); U = w("u")
    C = w("c"); M0 = w("m0")
    nc.vector.tensor_sub(DL[:], beta, alpha)                      # beta-alpha
    nc.vector.tensor_scalar(G2[:], GT[:], 2.0, 1e-30, ALU.mult, ALU.add)
    nc.vector.reciprocal(R2[:], G2[:])
    nc.vector.tensor_mul(Z[:], DL[:], R2[:])                      # zeta
    nc.vector.tensor_scalar_mul(NZ[:], Z[:], -1.0)
    nc.vector.tensor_max(AZ[:], Z[:], NZ[:])                      # |zeta|
    nc.vector.tensor_scalar_min(AZ[:], AZ[:], 3e17)
    nc.vector.tensor_mul(SQ[:], AZ[:], AZ[:])
    nc.scalar.activation(RHO[:], SQ[:], ACT.Sqrt, bias=1.0)       # sqrt(1+z^2)
    nc.vector.tensor_add(DEN[:], AZ[:], RHO[:])
    nc.vector.reciprocal(RD[:], DEN[:])
    nc.scalar.activation(SGN[:], Z[:], ACT.Sign)
    nc.vector.tensor_mul(T[:], SGN[:], RD[:])                     # tangent
    # c = rsqrt(1+t^2) with 2 Newton refinements (ACT rsqrt is low precision)
    nc.vector.tensor_mul(T2[:], T[:], T[:])
    nc.vector.tensor_scalar_add(A1[:], T2[:], 1.0)
    nc.scalar.activation(U[:], A1[:], ACT.Sqrt)
    nc.vector.reciprocal(C[:], U[:])
    for _ in range(2):
        nc.vector.tensor_mul(U[:], C[:], C[:])
        nc.vector.tensor_mul(U[:], U[:], A1[:])
        nc.vector.tensor_scalar(U[:], U[:], -0.5, 1.5, ALU.mult, ALU.add)
        nc.vector.tensor_mul(C[:], C[:], U[:])
    # incremental norms: alpha' = alpha - t*g, beta' = beta + t*g
    nc.vector.tensor_mul(M0[:], T[:], GT[:])
    nc.vector.tensor_sub(alpha, alpha, M0[:])
    nc.vector.tensor_add(beta, beta, M0[:])
    return T, C


def _jacobi_round(nc, wp, E, cdim, p_off, p_dims, q_off, q_dims, m, npairs_j,
                  NT, a_off, a_dims, b_off, b_dims, tag):
    """One parallel Jacobi round on tensor E.
    p_dims/q_dims: free dims selecting the p/q columns, each (128, m*cdim)
    with traversal (pairidx, j): pairidx count m, j count cdim.
    """
    ea = E[:]
    colp = _hap(ea, p_off, p_dims)
    colq = _hap(ea, q_off, q_dims)
    ne = m * cdim
    M1 = wp.tile([P, ne], F32, tag=f"{tag}m1")
    # gamma = sum_j colp*colq
    nc.vector.tensor_mul(M1[:], colp, colq)
    GT = wp.tile([P, m], F32, tag=f"{tag}g")
    m1a = M1[:]
    nc.vector.tensor_reduce(
        GT[:], _hap(m1a, 0, [m1a.ap[0], [cdim, m], [1, cdim]]),
        mybir.AxisListType.X, ALU.add)
    T, C = _angle_ops(nc, wp, m, GT, NT, a_off, a_dims, b_off, b_dims, tag)
    # broadcast APs for t and c over j
    ta, ca = T[:], C[:]
    tB = _hap(ta, 0, [ta.ap[0], [1, m], [0, cdim]])
    cB = _hap(ca, 0, [ca.ap[0], [1, m], [0, cdim]])
    N1 = wp.tile([P, ne], F32, tag=f"{tag}n1")
    N2 = wp.tile([P, ne], F32, tag=f"{tag}n2")
    nc.vector.tensor_mul(N1[:], colq, tB)
    nc.vector.tensor_mul(N2[:], colp, tB)
    nc.vector.tensor_sub(colp, colp, N1[:])
    nc.vector.tensor_add(colq, colq, N2[:])
    nc.gpsimd.tensor_mul(colp, colp, cB)
    nc.gpsimd.tensor_mul(colq, colq, cB)


def _exact_norms(nc, wp, E, ncols, cdim, NT, tag):
    """NT[bb*ncols + c] = sum_j E[bb, c, j]^2 (ncols*8 <= 64)."""
    ne = 8 * ncols * cdim
    SQ = wp.tile([P, ne], F32, tag=f"{tag}sq")
    nc.vector.tensor_mul(SQ[:], E[:], E[:])
    sa = SQ[:]
    nc.vector.tensor_reduce(
        NT[:], _hap(sa, 0, [sa.ap[0], [cdim, 8 * ncols], [1, cdim]]),
        mybir.AxisListType.X, ALU.add)


def build_program():
    nc = bacc.Bacc("TRN2", target_bir_lowering=False, debug=False)
    sites = nc.dram_tensor("sites", (NS, B, D), F32, kind="ExternalInput")
    phi = nc.dram_tensor("phi", (B,), F32, kind="ExternalOutput")

    with tile.TileContext(nc) as tc:
        with (
            tc.tile_pool(name="persist", bufs=1) as pp,
            tc.tile_pool(name="ld", bufs=4) as ldp,
            tc.tile_pool(name="tpps", bufs=4, space=bass.MemorySpace.PSUM) as tpp,
            tc.tile_pool(name="gmps", bufs=4, space=bass.MemorySpace.PSUM) as gmp,
            tc.tile_pool(name="dram", bufs=1, space=bass.MemorySpace.DRAM) as drp,
            tc.tile_pool(name="work", bufs=2) as wp,
        ):
            ident = pp.tile([P, P], F32, tag="ident")
            masks.make_identity(nc, ident[:])
            TX = [pp.tile([P, NS * B], F32, tag=f"tx{h}", name=f"tx{h}")
                  for h in range(2)]

            # ---- Phase L: natural load + PE transpose to k-on-partitions ----
            for i in range(NS):
                for bt in range(8):
                    ln = ldp.tile([P, D], F32, tag="ln")
                    nc.sync.dma_start(ln[:], sites[i, bt * 128:(bt + 1) * 128, :])
                    for kh in range(2):
                        pst = tpp.tile([P, P], F32, tag="tp")
                        nc.tensor.transpose(
                            pst[:], ln[:, kh * 128:(kh + 1) * 128], ident[:])
                        txa = TX[kh][:]
                        dst = _hap(txa, bt * 128 * 16 + i,
                                   [txa.ap[0], [16, 128]])
                        if bt % 2 == 0:
                            nc.vector.tensor_copy(dst, pst[:])
                        else:
                            nc.scalar.copy(dst, pst[:])

            # ---- Phase C: Gram matmuls (M=96: u=0..12 x bb=8, N=96: bb x v=4..16)
            gscr = drp.tile([NG, 16384], F32, tag="gscr")
            for g in range(NG):
                gt = gmp.tile([128, 128], F32, tag="gm")
                for kh in range(2):
                    blk = TX[kh][:, g * 128:(g + 1) * 128]
                    nc.tensor.matmul(gt[:], blk, blk,
                                     start=(kh == 0), stop=(kh == 1))
                gs = ldp.tile([128, 128], F32, tag="gs", name="gs")
                if g % 2 == 0:
                    nc.vector.tensor_copy(gs[:], gt[:])
                else:
                    nc.scalar.copy(gs[:], gt[:])
                nc.sync.dma_start(gscr[g:g + 1, :], gs[:])

            # ---- Phase X: DRAM->DRAM diagonal-block gathers ----
            # scratch flat idx (per g): u*768 + bb*108 + (v-4)   [m=(u,bb), n=(bb,v-4)]
            wscr = drp.tile([NG, 512], F32, tag="wscr")
            escr = drp.tile([NG, 256], F32, tag="escr")
            ga, wa, ea2 = gscr[:], wscr[:], escr[:]
            for g in range(NG):
                base = g * 16384
                # scratch flat (per g): bb*2064 + u*128 + v
                # C_w: u=i, v=8+j -> wscr[g, bb*64 + i*8 + j]
                nc.sync.dma_start(
                    _hap(wa, g * 512, [[64, 8], [8, 8], [1, 8]]),
                    _hap(ga, base + 8, [[2064, 8], [128, 8], [1, 8]]))
                # C_l: u=i, v=4+j -> escr[g, bb*32 + i*4 + j]
                nc.sync.dma_start(
                    _hap(ea2, g * 256, [[32, 8], [4, 4], [1, 4]]),
                    _hap(ga, base + 4, [[2064, 8], [128, 4], [1, 4]]))
                # C_r: u=8+i, v=12+j -> escr[g, bb*32 + 16 + i*4 + j]
                nc.sync.dma_start(
                    _hap(ea2, g * 256 + 16, [[32, 8], [4, 4], [1, 4]]),
                    _hap(ga, base + 8 * 128 + 12, [[2064, 8], [128, 4], [1, 4]]))

            # ---- read back entry-major ----
            EW = pp.tile([P, 512], F32, tag="ew")   # [g, bb*64 + c*8 + j]
            E2 = pp.tile([P, 256], F32, tag="e2")   # [g, bb*32 + blk*16 + c*4 + j]
            nc.sync.dma_start(EW[:], wscr[:])
            nc.sync.dma_start(E2[:], escr[:])

            # ---- Phase J: one-sided Jacobi ----
            NW = pp.tile([P, 64], F32, tag="nw")    # norms [bb*8 + c]
            N2T = pp.tile([P, 64], F32, tag="n2t")  # norms [bb*8 + blk*4 + c]
            _exact_norms(nc, wp, EW, 8, 8, NW, "xw")
            _exact_norms(nc, wp, E2, 8, 4, N2T, "x2")

            # E_W: music-chairs, fixed pairs (0,1),(2,3),(4,5),(6,7)
            # norms NW alpha: [bb*8 + 2a] -> dims [[2,32]], beta off 1
            ping = EW
            pong = pp.tile([P, 512], F32, tag="ew2")
            nwping, nwpong = NW, pp.tile([P, 64], F32, tag="nw2")
            n_rounds_w = N_SWEEP_W * 7
            for r in range(n_rounds_w):
                _jacobi_round(nc, wp, ping, 8,
                              0, [ping[:].ap[0], [16, 32], [1, 8]],
                              8, [ping[:].ap[0], [16, 32], [1, 8]],
                              32, 4, nwping, 0, [nwping[:].ap[0], [2, 32]],
                              1, [nwping[:].ap[0], [2, 32]], "w")
                if r < n_rounds_w - 1:
                    # permute slots: ring 2->4->6->7->5->3->1->2, 0 fixed
                    # dst <- src (slot contents move): new[sigma(s)] = old[s]
                    pa, qa = ping[:], pong[:]
                    npi, npo = nwping[:], nwpong[:]
                    for dsts, srcs in (((0,), (0,)), ((4, 6), (2, 4)),
                                       ((7,), (6,)), ((5, 3), (7, 5)),
                                       ((1,), (3,)), ((2,), (1,))):
                        n = len(dsts)
                        ds = (dsts[1] - dsts[0]) * 8 if n > 1 else 8
                        ss = (srcs[1] - srcs[0]) * 8 if n > 1 else 8
                        nc.vector.tensor_copy(
                            _hap(qa, dsts[0] * 8, [qa.ap[0], [64, 8], [ds, n], [1, 8]]),
                            _hap(pa, srcs[0] * 8, [pa.ap[0], [64, 8], [ss, n], [1, 8]]))
                        dn = (dsts[1] - dsts[0]) if n > 1 else 1
                        sn = (srcs[1] - srcs[0]) if n > 1 else 1
                        nc.scalar.copy(
                            _hap(npo, dsts[0], [npo.ap[0], [8, 8], [dn, n]]),
                            _hap(npi, srcs[0], [npi.ap[0], [8, 8], [sn, n]]))
                    ping, pong = pong, ping
                    nwping, nwpong = nwpong, nwping

            # E2: 3 round types, no permutation; pairs per block
            # cols at blk*16 + c*4; norms [bb*8 + blk*4 + c]
            e2a = E2[:]
            n2a = N2T[:]
            rounds2 = [
                # (p_off, p_cdims, q_off, q_cdims, na_off, na_dims, nb_off, nb_dims)
                (0, [8, 2], 4, [8, 2], 0, [2, 2], 1, [2, 2]),     # (0,1),(2,3)
                (0, [4, 2], 8, [4, 2], 0, [1, 2], 2, [1, 2]),     # (0,2),(1,3)
                (0, [4, 2], 12, [-4, 2], 0, [1, 2], 3, [-1, 2]),  # (0,3),(1,2)
            ]
            for sw in range(N_SWEEP_LR):
                for (po, pd, qo, qd, nao, nad, nbo, nbd) in rounds2:
                    _jacobi_round(
                        nc, wp, E2, 4,
                        po, [e2a.ap[0], [16, 16], pd, [1, 4]],
                        qo, [e2a.ap[0], [16, 16], qd, [1, 4]],
                        32, 4, N2T,
                        nao, [n2a.ap[0], [4, 16], nad],
                        nbo, [n2a.ap[0], [4, 16], nbd], "e")

            # ---- entropy ----
            EWf, NWf = ping, nwping
            LAMW = pp.tile([P, 64], F32, tag="lamw")
            LAM2 = pp.tile([P, 64], F32, tag="lam2")
            _exact_norms(nc, wp, EWf, 8, 8, LAMW, "fw")
            _exact_norms(nc, wp, E2, 8, 4, LAM2, "f2")

            TW = wp.tile([P, 8], F32, tag="tw")
            TL = wp.tile([P, 8], F32, tag="tl")
            TR = wp.tile([P, 8], F32, tag="tr")
            la, l2 = LAMW[:], LAM2[:]
            nc.vector.tensor_reduce(
                TW[:], _hap(la, 0, [la.ap[0], [8, 8], [1, 8]]),
                mybir.AxisListType.X, ALU.add)
            nc.vector.tensor_reduce(
                TL[:], _hap(l2, 0, [l2.ap[0], [8, 8], [1, 4]]),
                mybir.AxisListType.X, ALU.add)
            nc.vector.tensor_reduce(
                TR[:], _hap(l2, 4, [l2.ap[0], [8, 8], [1, 4]]),
                mybir.AxisListType.X, ALU.add)
            for TT_ in (TW, TL, TR):
                nc.vector.tensor_scalar_add(TT_[:], TT_[:], 1e-8)
                nc.vector.reciprocal(TT_[:], TT_[:])
            # lambda-hat
            LHW = pp.tile([P, 64], F32, tag="lhw")
            LH2 = pp.tile([P, 64], F32, tag="lh2")
            twa, tla, tra = TW[:], TL[:], TR[:]
            nc.vector.tensor_mul(LHW[:], LAMW[:],
                                 _hap(twa, 0, [twa.ap[0], [1, 8], [0, 8]]))
            lh2a = LH2[:]
            nc.vector.tensor_mul(
                _hap(lh2a, 0, [lh2a.ap[0], [8, 8], [1, 4]]),
                _hap(l2, 0, [l2.ap[0], [8, 8], [1, 4]]),
                _hap(tla, 0, [tla.ap[0], [1, 8], [0, 4]]))
            nc.vector.tensor_mul(
                _hap(lh2a, 4, [lh2a.ap[0], [8, 8], [1, 4]]),
                _hap(l2, 4, [l2.ap[0], [8, 8], [1, 4]]),
                _hap(tra, 0, [tra.ap[0], [1, 8], [0, 4]]))
            LGW = pp.tile([P, 64], F32, tag="lgw")
            LG2 = pp.tile([P, 64], F32, tag="lg2")
            EPSB = wp.tile([P, 1], F32, tag="epsb")
            nc.vector.memset(EPSB[:], 1e-10)
            nc.scalar.activation(LGW[:], LHW[:], ACT.Ln, bias=EPSB[:])
            nc.scalar.activation(LG2[:], LH2[:], ACT.Ln, bias=EPSB[:])
            nc.vector.tensor_mul(LGW[:], LGW[:], LHW[:])
            nc.vector.tensor_mul(LG2[:], LG2[:], LH2[:])
            SW = wp.tile([P, 8], F32, tag="sw")
            SL = wp.tile([P, 8], F32, tag="sl")
            SR = wp.tile([P, 8], F32, tag="sr")
            lgwa, lg2a = LGW[:], LG2[:]
            nc.vector.tensor_reduce(
                SW[:], _hap(lgwa, 0, [lgwa.ap[0], [8, 8], [1, 8]]),
                mybir.AxisListType.X, ALU.add)
            nc.vector.tensor_reduce(
                SL[:], _hap(lg2a, 0, [lg2a.ap[0], [8, 8], [1, 4]]),
                mybir.AxisListType.X, ALU.add)
            nc.vector.tensor_reduce(
                SR[:], _hap(lg2a, 4, [lg2a.ap[0], [8, 8], [1, 4]]),
                mybir.AxisListType.X, ALU.add)
            # phi = relu((SL + SR) - SW)   [entropies are negatives of sums]
            PHI = wp.tile([P, 8], F32, tag="phi")
            nc.vector.tensor_add(PHI[:], SL[:], SR[:])
            nc.vector.tensor_sub(PHI[:], PHI[:], SW[:])
            nc.vector.tensor_scalar_max(PHI[:], PHI[:], 0.0)
            # batch b = p*8 + bb
            pa = PHI[:]
            nc.sync.dma_start(phi[:].rearrange("(p b) -> p b", p=128), pa)

    nc.compile()
    return nc


_cached = None


def _get_program():
    global _cached
    if _cached is None:
        _cached = build_program()
    return _cached


def kernel(sites: np.ndarray) -> np.ndarray:
    assert sites.shape == (NS, NCORES * B, D), sites.shape
    nc = _get_program()
    in_maps = [
        {"sites": np.ascontiguousarray(sites[:, c * B:(c + 1) * B, :],
                                       dtype=np.float32)}
        for c in range(NCORES)
    ]
    res = run_bass_kernel_spmd(nc, in_maps, core_ids=list(range(NCORES)))
    out = np.concatenate([res.results[c]["phi"] for c in range(NCORES)])
    return out.astype(np.float32)


# revision 11
# speedup vs baseline: 1.2919x; 1.2919x over previous
"""PhiQ kernel for Trainium2: batched entanglement entropy via
PE Gram matmuls + one-sided Jacobi SVD on the vector engine.

Self-contained: kernel(sites) -> (8192,) float32.
Data-parallel over 8 cores (1024 batch each).
"""
import numpy as np
import concourse.bass as bass
import concourse.bacc as bacc
import concourse.mybir as mybir
import concourse.tile as tile
from concourse import masks
from concourse.ap import AP
from concourse.bass_utils import run_bass_kernel_spmd

F32 = mybir.dt.float32
ALU = mybir.AluOpType
ACT = mybir.ActivationFunctionType

NCORES = 8
B = 1024            # batch per core
NG = 128            # gram groups of 8 batches
NS, D = 16, 256
P = 128

N_SWEEP_W = 4       # 8x8 whole problem
N_SWEEP_LR = 3      # 4x4 left/right problems


def _hap(t_ap, extra_off, dims):
    """hand-built AP on the same backing tensor as t_ap"""
    return AP(t_ap.tensor, t_ap.offset + extra_off, dims)


def _angle_ops(nc, wp, m, GT, NT, a_off, a_dims, b_off, b_dims, tag):
    """Compute rotation params from gamma tile GT (128,m) and norms tile NT
    slices (alpha at a_off/a_dims, beta at b_off/b_dims).
    Returns (T, C) tiles (128, m): tangent and cosine. Also updates norms:
    alpha -= t*gamma, beta += t*gamma."""
    w = lambda tg: wp.tile([P, m], F32, tag=f"{tag}{tg}", name=f"{tag}{tg}")
    nta = NT[:]
    alpha = _hap(nta, a_off, a_dims)
    beta = _hap(nta, b_off, b_dims)
    DL = w("dl"); G2 = w("g2"); R2 = w("r2"); Z = w("z"); NZ = w("nz")
    AZ = w("az"); SQ = w("sq"); RHO = w("rho"); DEN = w("den"); RD = w("rd")
    SGN = w("sgn"); T = w("t"); T2 = w("t2"); A1 = w("a1"); U = w("u")
    C = w("c"); M0 = w("m0")
    nc.vector.tensor_sub(DL[:], beta, alpha)                      # beta-alpha
    nc.vector.tensor_scalar(G2[:], GT[:], 2.0, 1e-30, ALU.mult, ALU.add)
    nc.vector.reciprocal(R2[:], G2[:])
    nc.vector.tensor_mul(Z[:], DL[:], R2[:])                      # zeta
    nc.vector.tensor_scalar_mul(NZ[:], Z[:], -1.0)
    nc.vector.tensor_max(AZ[:], Z[:], NZ[:])                      # |zeta|
    nc.vector.tensor_scalar_min(AZ[:], AZ[:], 3e17)
    nc.vector.tensor_mul(SQ[:], AZ[:], AZ[:])
    nc.scalar.activation(RHO[:], SQ[:], ACT.Sqrt, bias=1.0)       # sqrt(1+z^2)
    nc.vector.tensor_add(DEN[:], AZ[:], RHO[:])
    nc.vector.reciprocal(RD[:], DEN[:])
    nc.scalar.activation(SGN[:], Z[:], ACT.Sign)
    nc.vector.tensor_mul(T[:], SGN[:], RD[:])                     # tangent
    # c = rsqrt(1+t^2) with 2 Newton refinements (ACT rsqrt is low precision)
    nc.vector.tensor_mul(T2[:], T[:], T[:])
    nc.vector.tensor_scalar_add(A1[:], T2[:], 1.0)
    nc.scalar.activation(U[:], A1[:], ACT.Sqrt)
    nc.vector.reciprocal(C[:], U[:])
    for _ in range(2):
        nc.vector.tensor_mul(U[:], C[:], C[:])
        nc.vector.tensor_mul(U[:], U[:], A1[:])
        nc.vector.tensor_scalar(U[:], U[:], -0.5, 1.5, ALU.mult, ALU.add)
        nc.vector.tensor_mul(C[:], C[:], U[:])
    # incremental norms: alpha' = alpha - t*g, beta' = beta + t*g
    nc.vector.tensor_mul(M0[:], T[:], GT[:])
    nc.vector.tensor_sub(alpha, alpha, M0[:])
    nc.vector.tensor_add(beta, beta, M0[:])
    return T, C


def _jacobi_round(nc, wp, E, cdim, p_off, p_dims, q_off, q_dims, m, npairs_j,
                  NT, a_off, a_dims, b_off, b_dims, tag):
    """One parallel Jacobi round on tensor E.
    p_dims/q_dims: free dims selecting the p/q columns, each (128, m*cdim)
    with traversal (pairidx, j): pairidx count m, j count cdim.
    """
    ea = E[:]
    colp = _hap(ea, p_off, p_dims)
    colq = _hap(ea, q_off, q_dims)
    ne = m * cdim
    M1 = wp.tile([P, ne], F32, tag=f"{tag}m1")
    # gamma = sum_j colp*colq
    nc.vector.tensor_mul(M1[:], colp, colq)
    GT = wp.tile([P, m], F32, tag=f"{tag}g")
    m1a = M1[:]
    nc.vector.tensor_reduce(
        GT[:], _hap(m1a, 0, [m1a.ap[0], [cdim, m], [1, cdim]]),
        mybir.AxisListType.X, ALU.add)
    T, C = _angle_ops(nc, wp, m, GT, NT, a_off, a_dims, b_off, b_dims, tag)
    # broadcast APs for t and c over j
    ta, ca = T[:], C[:]
    tB = _hap(ta, 0, [ta.ap[0], [1, m], [0, cdim]])
    cB = _hap(ca, 0, [ca.ap[0], [1, m], [0, cdim]])
    N1 = wp.tile([P, ne], F32, tag=f"{tag}n1")
    N2 = wp.tile([P, ne], F32, tag=f"{tag}n2")
    nc.vector.tensor_mul(N1[:], colq, tB)
    nc.vector.tensor_mul(N2[:], colp, tB)
    nc.vector.tensor_sub(colp, colp, N1[:])
    nc.vector.tensor_add(colq, colq, N2[:])
    nc.gpsimd.tensor_mul(colp, colp, cB)
    nc.gpsimd.tensor_mul(colq, colq, cB)


def _exact_norms(nc, wp, E, ncols, cdim, NT, tag):
    """NT[bb*ncols + c] = sum_j E[bb, c, j]^2 (ncols*8 <= 64)."""
    ne = 8 * ncols * cdim
    SQ = wp.tile([P, ne], F32, tag=f"{tag}sq")
    nc.vector.tensor_mul(SQ[:], E[:], E[:])
    sa = SQ[:]
    nc.vector.tensor_reduce(
        NT[:], _hap(sa, 0, [sa.ap[0], [cdim, 8 * ncols], [1, cdim]]),
        mybir.AxisListType.X, ALU.add)


def build_program():
    nc = bacc.Bacc("TRN2", target_bir_lowering=False, debug=False)
    sites = nc.dram_tensor("sites", (NS, B, D), F32, kind="ExternalInput")
    phi = nc.dram_tensor("phi", (B,), F32, kind="ExternalOutput")

    with tile.TileContext(nc) as tc:
        with (
            tc.tile_pool(name="persist", bufs=1) as pp,
            tc.tile_pool(name="ld", bufs=4) as ldp,
            tc.tile_pool(name="tpps", bufs=4, space=bass.MemorySpace.PSUM) as tpp,
            tc.tile_pool(name="gmps", bufs=4, space=bass.MemorySpace.PSUM) as gmp,
            tc.tile_pool(name="dram", bufs=1, space=bass.MemorySpace.DRAM) as drp,
            tc.tile_pool(name="work", bufs=2) as wp,
        ):
            ident = pp.tile([P, P], F32, tag="ident")
            masks.make_identity(nc, ident[:])
            TX = [pp.tile([P, NS * B], mybir.dt.float32r, tag=f"tx{h}", name=f"tx{h}")
                  for h in range(2)]

            # ---- Phase L: natural load + PE transpose to k-on-partitions ----
            for i in range(NS):
                for bt in range(8):
                    ln = ldp.tile([P, D], F32, tag="ln")
                    nc.sync.dma_start(ln[:], sites[i, bt * 128:(bt + 1) * 128, :])
                    for kh in range(2):
                        pst = tpp.tile([P, P], F32, tag="tp")
                        nc.tensor.transpose(
                            pst[:], ln[:, kh * 128:(kh + 1) * 128], ident[:])
                        txa = TX[kh][:]
                        dst = _hap(txa, bt * 128 * 16 + i,
                                   [txa.ap[0], [16, 128]])
                        if bt % 2 == 0:
                            nc.vector.tensor_copy(dst, pst[:])
                        else:
                            nc.scalar.copy(dst, pst[:])

            # ---- Phase C: Gram matmuls (M=96: u=0..12 x bb=8, N=96: bb x v=4..16)
            gscr = drp.tile([NG, 16384], F32, tag="gscr")
            for g in range(NG):
                gt = gmp.tile([128, 128], F32, tag="gm")
                for kh in range(2):
                    blk = TX[kh][:, g * 128:(g + 1) * 128]
                    nc.tensor.matmul(gt[:], blk, blk,
                                     start=(kh == 0), stop=(kh == 1))
                gs = ldp.tile([128, 128], F32, tag="gs", name="gs")
                if g % 2 == 0:
                    nc.vector.tensor_copy(gs[:], gt[:])
                else:
                    nc.scalar.copy(gs[:], gt[:])
                nc.sync.dma_start(gscr[g:g + 1, :], gs[:])

            # ---- Phase X: one diagonal-superblock gather per group ----
            # scratch flat (per g): bb*2064 + u*128 + v
            # dscr[g, bb*256 + u*16 + v] = dense per-batch 16x16 gram
            dscr = drp.tile([NG, 2048], F32, tag="dscr")
            ga, da = gscr[:], dscr[:]
            qeng = [nc.sync, nc.gpsimd, nc.scalar]
            for g in range(NG):
                qeng[g % 3].dma_start(
                    _hap(da, g * 2048, [[256, 8], [16, 16], [1, 16]]),
                    _hap(ga, g * 16384, [[2064, 8], [128, 16], [1, 16]]))

            # ---- read back dense per-batch grams, extract on DVE ----
            ED = pp.tile([P, 2048], F32, tag="ed")
            nc.sync.dma_start(ED[:], dscr[:])
            EW = pp.tile([P, 512], F32, tag="ew")   # [g, bb*64 + c*8 + j]
            E2 = pp.tile([P, 256], F32, tag="e2")   # [g, bb*32 + blk*16 + c*4 + j]
            eda, ewa, e2b = ED[:], EW[:], E2[:]
            nc.vector.tensor_copy(
                _hap(ewa, 0, [ewa.ap[0], [64, 8], [8, 8], [1, 8]]),
                _hap(eda, 8, [eda.ap[0], [256, 8], [16, 8], [1, 8]]))
            nc.vector.tensor_copy(
                _hap(e2b, 0, [e2b.ap[0], [32, 8], [4, 4], [1, 4]]),
                _hap(eda, 4, [eda.ap[0], [256, 8], [16, 4], [1, 4]]))
            nc.vector.tensor_copy(
                _hap(e2b, 16, [e2b.ap[0], [32, 8], [4, 4], [1, 4]]),
                _hap(eda, 140, [eda.ap[0], [256, 8], [16, 4], [1, 4]]))

            # ---- Phase J: one-sided Jacobi ----
            NW = pp.tile([P, 64], F32, tag="nw")    # norms [bb*8 + c]
            N2T = pp.tile([P, 64], F32, tag="n2t")  # norms [bb*8 + blk*4 + c]
            _exact_norms(nc, wp, EW, 8, 8, NW, "xw")
            _exact_norms(nc, wp, E2, 8, 4, N2T, "x2")

            # E_W: music-chairs, fixed pairs (0,1),(2,3),(4,5),(6,7)
            # norms NW alpha: [bb*8 + 2a] -> dims [[2,32]], beta off 1
            ping = EW
            pong = pp.tile([P, 512], F32, tag="ew2")
            nwping, nwpong = NW, pp.tile([P, 64], F32, tag="nw2")
            n_rounds_w = N_SWEEP_W * 7
            for r in range(n_rounds_w):
                _jacobi_round(nc, wp, ping, 8,
                              0, [ping[:].ap[0], [16, 32], [1, 8]],
                              8, [ping[:].ap[0], [16, 32], [1, 8]],
                              32, 4, nwping, 0, [nwping[:].ap[0], [2, 32]],
                              1, [nwping[:].ap[0], [2, 32]], "w")
                if r < n_rounds_w - 1:
                    # permute slots: ring 2->4->6->7->5->3->1->2, 0 fixed
                    # dst <- src (slot contents move): new[sigma(s)] = old[s]
                    pa, qa = ping[:], pong[:]
                    npi, npo = nwping[:], nwpong[:]
                    for dsts, srcs in (((0,), (0,)), ((4, 6), (2, 4)),
                                       ((7,), (6,)), ((5, 3), (7, 5)),
                                       ((1,), (3,)), ((2,), (1,))):
                        n = len(dsts)
                        ds = (dsts[1] - dsts[0]) * 8 if n > 1 else 8
                        ss = (srcs[1] - srcs[0]) * 8 if n > 1 else 8
                        nc.gpsimd.tensor_copy(
                            _hap(qa, dsts[0] * 8, [qa.ap[0], [64, 8], [ds, n], [1, 8]]),
                            _hap(pa, srcs[0] * 8, [pa.ap[0], [64, 8], [ss, n], [1, 8]]))
                        dn = (dsts[1] - dsts[0]) if n > 1 else 1
                        sn = (srcs[1] - srcs[0]) if n > 1 else 1
                        nc.scalar.copy(
                            _hap(npo, dsts[0], [npo.ap[0], [8, 8], [dn, n]]),
                            _hap(npi, srcs[0], [npi.ap[0], [8, 8], [sn, n]]))
                    ping, pong = pong, ping
                    nwping, nwpong = nwpong, nwping

            # E2: 3 round types, no permutation; pairs per block
            # cols at blk*16 + c*4; norms [bb*8 + blk*4 + c]
            e2a = E2[:]
            n2a = N2T[:]
            rounds2 = [
                # (p_off, p_cdims, q_off, q_cdims, na_off, na_dims, nb_off, nb_dims)
                (0, [8, 2], 4, [8, 2], 0, [2, 2], 1, [2, 2]),     # (0,1),(2,3)
                (0, [4, 2], 8, [4, 2], 0, [1, 2], 2, [1, 2]),     # (0,2),(1,3)
                (0, [4, 2], 12, [-4, 2], 0, [1, 2], 3, [-1, 2]),  # (0,3),(1,2)
            ]
            for sw in range(N_SWEEP_LR):
                for (po, pd, qo, qd, nao, nad, nbo, nbd) in rounds2:
                    _jacobi_round(
                        nc, wp, E2, 4,
                        po, [e2a.ap[0], [16, 16], pd, [1, 4]],
                        qo, [e2a.ap[0], [16, 16], qd, [1, 4]],
                        32, 4, N2T,
                        nao, [n2a.ap[0], [4, 16], nad],
                        nbo, [n2a.ap[0], [4, 16], nbd], "e")

            # ---- entropy ----
            EWf, NWf = ping, nwping
            LAMW = pp.tile([P, 64], F32, tag="lamw")
            LAM2 = pp.tile([P, 64], F32, tag="lam2")
            _exact_norms(nc, wp, EWf, 8, 8, LAMW, "fw")
            _exact_norms(nc, wp, E2, 8, 4, LAM2, "f2")

            TW = wp.tile([P, 8], F32, tag="tw")
            TL = wp.tile([P, 8], F32, tag="tl")
            TR = wp.tile([P, 8], F32, tag="tr")
            la, l2 = LAMW[:], LAM2[:]
            nc.vector.tensor_reduce(
                TW[:], _hap(la, 0, [la.ap[0], [8, 8], [1, 8]]),
                mybir.AxisListType.X, ALU.add)
            nc.vector.tensor_reduce(
                TL[:], _hap(l2, 0, [l2.ap[0], [8, 8], [1, 4]]),
                mybir.AxisListType.X, ALU.add)
            nc.vector.tensor_reduce(
                TR[:], _hap(l2, 4, [l2.ap[0], [8, 8], [1, 4]]),
                mybir.AxisListType.X, ALU.add)
            for TT_ in (TW, TL, TR):
                nc.vector.tensor_scalar_add(TT_[:], TT_[:], 1e-8)
                nc.vector.reciprocal(TT_[:], TT_[:])
            # lambda-hat
            LHW = pp.tile([P, 64], F32, tag="lhw")
            LH2 = pp.tile([P, 64], F32, tag="lh2")
            twa, tla, tra = TW[:], TL[:], TR[:]
            nc.vector.tensor_mul(LHW[:], LAMW[:],
                                 _hap(twa, 0, [twa.ap[0], [1, 8], [0, 8]]))
            lh2a = LH2[:]
            nc.vector.tensor_mul(
                _hap(lh2a, 0, [lh2a.ap[0], [8, 8], [1, 4]]),
                _hap(l2, 0, [l2.ap[0], [8, 8], [1, 4]]),
                _hap(tla, 0, [tla.ap[0], [1, 8], [0, 4]]))
            nc.vector.tensor_mul(
                _hap(lh2a, 4, [lh2a.ap[0], [8, 8], [1, 4]]),
                _hap(l2, 4, [l2.ap[0], [8, 8], [1, 4]]),
                _hap(tra, 0, [tra.ap[0], [1, 8], [0, 4]]))
            LGW = pp.tile([P, 64], F32, tag="lgw")
            LG2 = pp.tile([P, 64], F32, tag="lg2")
            EPSB = wp.tile([P, 1], F32, tag="epsb")
            nc.vector.memset(EPSB[:], 1e-10)
            nc.scalar.activation(LGW[:], LHW[:], ACT.Ln, bias=EPSB[:])
            nc.scalar.activation(LG2[:], LH2[:], ACT.Ln, bias=EPSB[:])
            nc.vector.tensor_mul(LGW[:], LGW[:], LHW[:])
            nc.vector.tensor_mul(LG2[:], LG2[:], LH2[:])
            SW = wp.tile([P, 8], F32, tag="sw")
            SL = wp.tile([P, 8], F32, tag="sl")
            SR = wp.tile([P, 8], F32, tag="sr")
            lgwa, lg2a = LGW[:], LG2[:]
            nc.vector.tensor_reduce(
                SW[:], _hap(lgwa, 0, [lgwa.ap[0], [8, 8], [1, 8]]),
                mybir.AxisListType.X, ALU.add)
            nc.vector.tensor_reduce(
                SL[:], _hap(lg2a, 0, [lg2a.ap[0], [8, 8], [1, 4]]),
                mybir.AxisListType.X, ALU.add)
            nc.vector.tensor_reduce(
                SR[:], _hap(lg2a, 4, [lg2a.ap[0], [8, 8], [1, 4]]),
                mybir.AxisListType.X, ALU.add)
            # phi = relu((SL + SR) - SW)   [entropies are negatives of sums]
            PHI = wp.tile([P, 8], F32, tag="phi")
            nc.vector.tensor_add(PHI[:], SL[:], SR[:])
            nc.vector.tensor_sub(PHI[:], PHI[:], SW[:])
            nc.vector.tensor_scalar_max(PHI[:], PHI[:], 0.0)
            # batch b = p*8 + bb
            pa = PHI[:]
            nc.sync.dma_start(phi[:].rearrange("(p b) -> p b", p=128), pa)

    nc.compile()
    return nc


_cached = None


def _get_program():
    global _cached
    if _cached is None:
        _cached = build_program()
    return _cached


def kernel(sites: np.ndarray) -> np.ndarray:
    assert sites.shape == (NS, NCORES * B, D), sites.shape
    nc = _get_program()
    in_maps = [
        {"sites": np.ascontiguousarray(sites[:, c * B:(c + 1) * B, :],
                                       dtype=np.float32)}
        for c in range(NCORES)
    ]
    res = run_bass_kernel_spmd(nc, in_maps, core_ids=list(range(NCORES)))
    out = np.concatenate([res.results[c]["phi"] for c in range(NCORES)])
    return out.astype(np.float32)
